# revision 41
# baseline (speedup 1.0000x reference)
"""VQ codebook lookup (nn_VQ) on 8 TRN2 NeuronCores.

reference: idx = argmin_k ||x_n - e_k||^2 ; out = embeddings[idx]
Equivalent: idx = argmax_k (x_n . e_k - 0.5||e_k||^2)  (||x||^2 is constant per row)

Strategy v4 (data-parallel over N, codebook replicated):
  - Host: per core, shard x into [62500, 100] f32, pad to [63488, 100]
    (62 super-tiles of 1024 rows), pre-transpose each super-tile to
    [101, 1024] (row 100 = 1.0 bias-aug) with the 1024 columns stored in
    interleaved order (device col c*128+p holds original row 8p+c) so the
    output DMA sees contiguous multi-row elements, then split into bf16
    hi/lo halves: xt [62, 2, 101, 1024] bf16. Codebook constants:
    eth_h/eth_l [101, 100] bf16 hi/lo of [e.T; -0.5||e||^2],
    e2 [100, 100] bf16, identity [128, 128] bf16.
  - Device, per 1024-row super-tile:
      scores: 8 chunks x 3 accumulating bf16 matmuls (x_hi.eth_h + x_hi.eth_l
        + x_lo.eth_h) -> PSUM [128, 8, 128] f32 (chunks strided 512B to stay
        bank-aligned, 100 real cols each)
      DVE reduce_max + is_ge (broadcast AP) -> exact one-hot bf16 mask
      8x PE-transpose (bf16, identity moving) -> maskT PSUM [100, 8, 128]
      Act copy (f32-bitcast view, halves element count) -> maskTs SBUF
      gather: 8x bf16 matmul (maskT chunk stationary, e2 moving)
        -> out rows PSUM [128, 8, 128] f32 (512B-strided chunks)
      Act copy (casts f32->bf16) into a group buffer; per-tile DMA out
      (contiguous 1600B elems via the interleave).
  - DMAs: input loads grouped 2 super-tiles per instruction (plain loads of
    the host-pretransposed layout; elem = 2KB). Output bf16, upcast on host.
  - Software pipeline: per iteration i emit S(i), RM(i-1), C2(i-4), TC(i-2),
    G(i-3). S first keeps the score matmuls (which feed the DVE critical
    path) ahead of the gather's outp-buffer wait on the PE queue; with deep
    mask-side buffer rings (bufs=8) every steady-state period is exactly the
    DVE floor: reduce 958 + sem 160 + is_ge 958 = 2076 ns per tile.
    30 warmup matmuls on a memset scratch ramp the PE p-state during the
    initial DMAs so the first score tile runs at full clock; the constants
    load goes via SWDGE (gpsimd) so it never queues ahead of the first
    input tile on the HWDGE ring.
    The last super-tile holds only 36 real rows (62500 = 61*1024 + 36); it is
    stored un-interleaved so all real rows land in chunk 0 and the device
    processes just 1 of its 8 chunks, shrinking the pipeline drain chain.
"""

import sys

sys.path.insert(0, "/opt/trn_rl_repo")
from contextlib import ExitStack

import ml_dtypes
import numpy as np

import concourse.bass as bass
import concourse.bacc as bacc
import concourse.tile as tile
from concourse import mybir
from concourse._compat import with_exitstack
from concourse.bass_utils import run_bass_kernel_spmd

F32 = mybir.dt.float32
BF = mybir.dt.bfloat16
bf16 = ml_dtypes.bfloat16

N_TOTAL = 500_000
D = 100
K = 100
N_CORES = 8
ST = 1024  # rows per super-tile
NCH = ST // 128  # 8 chunks
N_SHARD = N_TOTAL // N_CORES  # 62500
T = -(-N_SHARD // ST)  # 62 super-tiles
NP = T * ST  # 63488 padded rows per core
GIN = 2  # super-tiles per input DMA
GOUT = 2  # super-tiles per output DMA
NG = T // GIN  # 31 groups


@with_exitstack
def _vq_tile_kernel(ctx: ExitStack, tc: tile.TileContext, out, xt, cb):
    nc = tc.nc

    consts = ctx.enter_context(tc.tile_pool(name="consts", bufs=1))
    cb_s = consts.tile([128, 428], BF, tag="cb")
    nc.gpsimd.dma_start(cb_s[:], cb[:])
    id_s = cb_s[:, 0:128]
    ethh_s = cb_s[0:101, 128:228]
    ethl_s = cb_s[0:101, 228:328]
    e2_s = cb_s[0:K, 328:428]

    xp = ctx.enter_context(tc.tile_pool(name="xt", bufs=4))
    sp = ctx.enter_context(tc.tile_pool(name="scores", bufs=2, space="PSUM"))
    mvp = ctx.enter_context(tc.tile_pool(name="maxv", bufs=4))
    mp = ctx.enter_context(tc.tile_pool(name="mask", bufs=4))
    mtp = ctx.enter_context(tc.tile_pool(name="maskT", bufs=2, space="PSUM"))
    msp = ctx.enter_context(tc.tile_pool(name="maskTs", bufs=4))
    opp = ctx.enter_context(tc.tile_pool(name="outp", bufs=1, space="PSUM"))
    ogp = ctx.enter_context(tc.tile_pool(name="outg", bufs=6))

    xt_v = xt.rearrange("(g u) d n -> g d u n", u=GIN)  # [31, 101, 2, 2048]
    out_v = out.rearrange("(g u p w) d -> g p u w d", u=GOUT, p=128, w=NCH)

    xtiles = {}
    scores_t = {}
    mask_t = {}
    maskTs_t = {}
    outp_t = {}
    outg_t = {}

    def load(g):
        tl = xp.tile([101, GIN, 2 * ST], BF, tag="xt")
        if g <= 1:
            for u in range(GIN):
                nc.sync.dma_start(out=tl[:, u], in_=xt_v[g, :, u])
        else:
            nc.sync.dma_start(out=tl[:], in_=xt_v[g])
        xtiles[g] = tl

    def S(t):
        g, u = divmod(t, GIN)
        tl = xtiles[g]
        nch = 1 if t == T - 1 else NCH
        sc = sp.tile([128, NCH, 128], F32, tag="scores")
        for c in range(nch):
            hi = tl[:, u, bass.ts(c, 128)]
            lo = tl[:, u, ST + c * 128 : ST + (c + 1) * 128]
            nc.tensor.matmul(sc[:, c, 0:K], hi, ethh_s, start=True, stop=False)
            nc.tensor.matmul(sc[:, c, 0:K], hi, ethl_s, start=False, stop=False)
            nc.tensor.matmul(sc[:, c, 0:K], lo, ethh_s, start=False, stop=True)
        scores_t[t] = sc
        if u == GIN - 1:
            del xtiles[g]

    def RM(t):
        nch = 1 if t == T - 1 else NCH
        sc = scores_t.pop(t)
        mv = mvp.tile([128, NCH], F32, tag="maxv")
        nc.vector.tensor_reduce(
            mv[:, 0:nch],
            sc[:, 0:nch, 0:K],
            axis=mybir.AxisListType.X,
            op=mybir.AluOpType.max,
        )
        mk = mp.tile([128, NCH, K], BF, tag="mask")
        mvv = mv[:, 0:nch].rearrange("p (f o) -> p f o", o=1)
        s_ap, m_ap = bass.broadcast_tensor_aps(sc[:, 0:nch, 0:K], mvv)
        nc.vector.tensor_tensor(
            out=mk[:, 0:nch], in0=s_ap, in1=m_ap, op=mybir.AluOpType.is_ge
        )
        mask_t[t] = mk

    def TC(t):
        nch = 1 if t == T - 1 else NCH
        mk = mask_t.pop(t)
        mt = mtp.tile([K, NCH, 128], BF, tag="maskT")
        for c in range(nch):
            nc.tensor.transpose(mt[:, c], mk[:, c], id_s)
        ms = msp.tile([K, NCH, 128], BF, tag="maskTs")
        nc.scalar.copy(ms[:, 0:nch].bitcast(F32), mt[:, 0:nch].bitcast(F32))
        maskTs_t[t] = ms

    def G(t):
        nch = 1 if t == T - 1 else NCH
        ms = maskTs_t.pop(t)
        op_ = opp.tile([128, NCH, 128], F32, tag="outp")
        for c in range(nch):
            nc.tensor.matmul(op_[:, c, 0:D], ms[:, c], e2_s, start=True, stop=True)
        outp_t[t] = op_

    def C2(t):
        g, u = divmod(t, GOUT)
        if u == 0:
            outg_t[g] = ogp.tile([128, GOUT, NCH, D], BF, tag="outg", name="outg")
        og = outg_t[g]
        op_ = outp_t.pop(t)
        if t == T - 1:
            # tail tile: un-interleaved, 1 chunk; rows (T-1)*ST + p
            nc.scalar.copy(og[:, u, 0], op_[:, 0, 0:D])
            nc.sync.dma_start(
                out=out[(T - 1) * ST : (T - 1) * ST + 128], in_=og[:, u, 0]
            )
        else:
            nc.scalar.copy(og[:, u], op_[:, :, 0:D])
            nc.sync.dma_start(out=out_v[g, :, u], in_=og[:, u])
        if u == GOUT - 1:
            del outg_t[g]

    load(0)
    load(1)
    for i in range(T + 4):
        if i % GIN == 0:
            g = i // GIN + 2
            if g < NG:
                load(g)
        if 0 <= i - 2 < T:
            TC(i - 2)
        if 0 <= i - 3 < T:
            G(i - 3)
        if 0 <= i - 4 < T:
            C2(i - 4)
        if i < T:
            S(i)
        if 0 <= i - 1 < T:
            RM(i - 1)


def build_nc():
    nc = bacc.Bacc(
        "TRN2",
        target_bir_lowering=False,
        debug=False,
        enable_asserts=False,
        num_devices=N_CORES,
    )
    out = nc.dram_tensor("out", [NP, D], BF, kind="ExternalOutput").ap()
    xt = nc.dram_tensor("xt", [T, 101, 2 * ST], BF, kind="ExternalInput").ap()
    cb = nc.dram_tensor("cb", [128, 428], BF, kind="ExternalInput").ap()
    with tile.TileContext(nc) as tc:
        _vq_tile_kernel(tc, out, xt, cb)
    nc.compile()
    return nc


def prep_inputs(inputs: np.ndarray, embeddings: np.ndarray):
    """Host-side shard + layout prep. Returns in_maps for the 8 cores."""
    x = np.ascontiguousarray(inputs, dtype=np.float32)
    e = np.ascontiguousarray(embeddings, dtype=np.float32)

    ethf = np.zeros((101, K), dtype=np.float32)
    ethf[0:D] = e.T
    ethf[D] = (-0.5 * np.sum(e.astype(np.float64) ** 2, axis=1)).astype(np.float32)
    ethh = ethf.astype(bf16)
    ethl = (ethf - ethh.astype(np.float32)).astype(bf16)
    cb = np.zeros((128, 428), dtype=bf16)
    cb[:, 0:128] = np.eye(128, dtype=bf16)
    cb[0:101, 128:228] = ethh
    cb[0:101, 228:328] = ethl
    cb[0:K, 328:428] = e.astype(bf16)

    in_maps = []
    for i in range(N_CORES):
        xs = x[i * N_SHARD : (i + 1) * N_SHARD]
        xpad = np.zeros((NP, D), dtype=np.float32)
        xpad[:N_SHARD] = xs
        # device col c*128+p holds original row NCH*p+c of the super-tile
        v = xpad.reshape(T, 128, NCH, D)
        xtf = np.empty((T, 101, ST), dtype=np.float32)
        xtf[:, 0:D, :] = v.transpose(0, 3, 2, 1).reshape(T, D, ST)
        # tail tile: no interleave so all real rows land in chunk 0
        xtf[T - 1, 0:D, :] = xpad[(T - 1) * ST :].T
        xtf[:, D, :] = 1.0
        xt = np.empty((T, 101, 2 * ST), dtype=bf16)
        xt[:, :, 0:ST] = xtf.astype(bf16)
        xt[:, :, ST:] = (xtf - xt[:, :, 0:ST].astype(np.float32)).astype(bf16)
        in_maps.append({"xt": xt, "cb": cb})
    return in_maps


_NC_CACHE = None


def kernel(inputs: np.ndarray, embeddings: np.ndarray) -> np.ndarray:
    global _NC_CACHE
    if _NC_CACHE is None:
        _NC_CACHE = build_nc()
    nc = _NC_CACHE
    in_maps = prep_inputs(inputs, embeddings)
    res = run_bass_kernel_spmd(nc, in_maps, core_ids=list(range(N_CORES)))
    shards = [res.results[i]["out"][:N_SHARD] for i in range(N_CORES)]
    full = np.concatenate(shards, axis=0)
    return np.ascontiguousarray(full.astype(np.float32))
